# revision 8
# baseline (speedup 1.0000x reference)
"""Trainium2 Bass kernel for nn_CrossAttention_12953621365289.

Self-attention with q=k=v (faithful to the reference's to_q-overwrite bug).
Full inputs in, full output out; internally sharded across 8 NeuronCores as
(batch, head-group):

    core c -> batch b = c//2, heads hg = c%2 (8 heads each)

Each core computes qkv for ONLY its 8 heads' channels (half the projection
work of a query-split layout), runs attention for its heads over all 2048
queries, and produces a PARTIAL output projection y_part = o_hg @ W_out[hg];
the host sums the two partials per batch (and the bias rides along on the
hg=0 core).  No cross-core collectives.

Phase B math tricks (per head):
  - scores computed directly in [keys, queries] orientation via the symmetric
    q @ q^T, keys on partitions; attn @ v runs without transposes and the
    ones-column of v_aug accumulates the softmax denominator Z (row 64).
  - softmax skips max-subtraction (logits bounded ~17) and normalizes the
    output: o[:, i] /= Z_i.
  - exp is split across TWO engines: scalar-engine ACTIVATE(Exp) for ~5/8 of
    the (head, jb) score tiles, and a Schraudolph bit-trick exp on the vector
    engine for the rest (tensor_scalar mult+add -> int32, bitcast to f32:
    e^x ~ bitcast(A*x + B)); measured end-to-end rel err ~4e-3, well inside
    the 2e-2 gate.  This halves the scalar-engine exp wall (~294us if
    scalar-only, which would dominate the kernel).
  - score matmuls for the two heads of a partition block are interleaved
    instruction-by-instruction: they sit on disjoint PE row tiles (64x128
    tiles T0/T8, inferred from partition offsets), which the PE can co-execute.

All matmul operands are float32r (fp32 data, relaxed-precision PE mode).
"""

import sys

if '/opt/trn_rl_repo' not in sys.path:
    sys.path.insert(0, '/opt/trn_rl_repo')

import numpy as np

import concourse.bass as bass
import concourse.tile as tile
from concourse import bacc, mybir
from concourse.bass_utils import run_bass_kernel_spmd

F32 = mybir.dt.float32
F32R = mybir.dt.float32r
I32 = mybir.dt.int32
AF = mybir.ActivationFunctionType
ALU = mybir.AluOpType

B, N, D = 4, 2048, 1024
H, DH = 16, 64
H8 = 8               # heads per core
DG = H8 * DH         # 512 qkv channels per core
SCALE = DH ** -0.5
N_CORES = 8
VW = DH + 1          # 65: v columns + ones column

# Schraudolph exp constants: e^s ~ bitcast_f32(int32(As*s + Bs)), with the
# attention scale folded into As.
_LOG2E = 1.4426950408889634
A_S = float((1 << 23) * _LOG2E * SCALE)
B_S = float(127 * (1 << 23) - 366393.0)
# (head-tile, jb) -> vector-engine Schraudolph instead of scalar ACTIVATE
VEC_JB = frozenset({2, 5, 7, 10, 13, 15})

_CACHE = {}


def _build_program(reps=1):
    nc = bacc.Bacc(name="cross_attn_hs")

    xT_ext = nc.declare_dram_parameter("xT", [D, N], F32, isOutput=False)
    wq_ext = nc.declare_dram_parameter("w_qkv", [D, DG], F32, isOutput=False)
    wo_ext = nc.declare_dram_parameter("w_out", [DG, D], F32, isOutput=False)
    bo_ext = nc.declare_dram_parameter("b_out", [1, D], F32, isOutput=False)
    y_ext = nc.declare_dram_parameter("y", [N, D], F32, isOutput=True)

    KB = D // 128     # 8 d_model partition blocks
    CB = DG // 128    # 4 qkv-channel partition blocks (2 heads each)
    TB = N // 128     # 16 token partition blocks

    with tile.TileContext(nc) as tc:
        with tc.tile_pool(name="persist", bufs=1) as persist, \
             tc.tile_pool(name="dram", bufs=1, space="DRAM") as dramp:
            # qkv^T: [channel, token] layout, channel-major head order
            qkvT = [persist.tile([128, N], F32R, tag=f"qkvT{k}",
                                 name=f"qkvT{k}") for k in range(CB)]
            # v in natural layout [token, head*65] with a ones column per head
            vaug = [persist.tile([128, H8 * VW], F32R, tag=f"vaug{t}",
                                 name=f"vaug{t}") for t in range(TB)]
            bias_sb = persist.tile([128, D], F32, tag="bias")
            ones_f32 = persist.tile([128, 64], F32, tag="onesf")
            ident_f32 = persist.tile([128, 128], F32, tag="identf")
            ident2 = persist.tile([128, 256], F32R, tag="ident2")
            # DRAM scratch for the softmax-denominator broadcast bounce
            zscr = dramp.tile([H8, N], F32, tag="zscr")

            nc.vector.memset(ones_f32[:], 1.0)
            from concourse.masks import make_identity
            make_identity(nc, ident_f32[:])
            nc.vector.tensor_copy(out=ident2[:, 0:128], in_=ident_f32[:])
            nc.vector.tensor_copy(out=ident2[:, 128:256], in_=ident_f32[:])
            # bias broadcast to all partitions (DRAM-source stride-0 DMA)
            nc.gpsimd.dma_start(
                out=bias_sb[:], in_=bo_ext[:].partition_broadcast(128))
            # ones columns of v_aug
            for t in range(TB):
                va3 = vaug[t][:, :].rearrange("p (h w) -> p h w", h=H8)
                nc.vector.tensor_copy(out=va3[:, :, DH], in_=ones_f32[:, 0:H8])

            # reps>1 repeats the whole body inside one NEFF (benchmark builds)
            for _rep in range(reps):
                # ------------- Phase A: qkvT = (x @ W_qkv[:, hg])^T --------
                with tc.tile_pool(name="wq", bufs=1) as wqp, \
                     tc.tile_pool(name="xtq", bufs=2) as xtp, \
                     tc.tile_pool(name="psA", bufs=4, space="PSUM") as psA, \
                     tc.tile_pool(name="psT", bufs=3, space="PSUM") as psT:
                    wqt = [wqp.tile([128, DG], F32R, tag=f"wq{k}",
                                    name=f"wq{k}") for k in range(KB)]
                    for k in range(KB):
                        nc.gpsimd.dma_start(
                            out=wqt[k][:], in_=wq_ext[k * 128:(k + 1) * 128, :])
                    for tq in range(4):     # token quarters (512 each)
                        ts = slice(tq * 512, (tq + 1) * 512)
                        xt = xtp.tile([128, KB, 512], F32R, tag="xtq")
                        for k in range(KB):
                            nc.gpsimd.dma_start(
                                out=xt[:, k, :],
                                in_=xT_ext[k * 128:(k + 1) * 128, ts])
                        for cb in range(CB):
                            ps = psA.tile([128, 512], F32, tag="ps")
                            for k in range(KB):
                                nc.tensor.matmul(
                                    out=ps[:],
                                    lhsT=wqt[k][:, cb * 128:(cb + 1) * 128],
                                    rhs=xt[:, k, :],
                                    start=(k == 0), stop=(k == KB - 1))
                            nc.vector.tensor_copy(out=qkvT[cb][:, ts], in_=ps[:])
                    # v_aug via identity matmuls, N=256 so fp32r streams at
                    # 1 cyc/row (N=128 runs 4x slower); t-outer so vaug[0]
                    # is ready first for phase B
                    for t in range(TB):
                        for cb in range(CB):
                            trp2 = psT.tile([128, 256], F32, tag="trp")
                            nc.tensor.matmul(
                                out=trp2[:],
                                lhsT=qkvT[cb][:, t * 128:(t + 1) * 128],
                                rhs=ident2[:],
                                start=True, stop=True)
                            va3 = vaug[t][:, :].rearrange(
                                "p (h w) -> p h w", h=H8)
                            nc.vector.tensor_copy(
                                out=va3[:, 2 * cb:2 * cb + 2, 0:DH],
                                in_=trp2[:, 0:128].rearrange(
                                    "p (h d) -> p h d", h=2))

                # ------------- Phase B: attention, head pairs --------------
                with tc.tile_pool(name="ohT", bufs=1) as ohp:
                    ohT = [ohp.tile([128, N], F32R, tag=f"ohT{k}",
                                    name=f"ohT{k}") for k in range(CB)]
                    with tc.tile_pool(name="e", bufs=6) as ep, \
                         tc.tile_pool(name="mS", bufs=2, space="PSUM") as psS, \
                         tc.tile_pool(name="mO", bufs=2, space="PSUM") as psO, \
                         tc.tile_pool(name="misc", bufs=4) as mp:
                        for pb in range(CB):        # head pair h0, h1
                            qh = [qkvT[pb][0:64, :], qkvT[pb][64:128, :]]
                            for q2 in range(2):     # query halves (1024)
                                qs = slice(q2 * 1024, (q2 + 1) * 1024)
                                oT = [psO.tile([VW, 1024], F32, tag="oT",
                                               name=f"oT{_hi}")
                                      for _hi in range(2)]
                                for jb in range(TB):
                                    sp = [psS.tile([128, 1024], F32, tag="s",
                                                   name=f"sp{_hi}")
                                          for _hi in range(2)]
                                    # interleave the two heads' score matmuls:
                                    # disjoint 64x128 PE row tiles co-execute
                                    for c5 in range(2):
                                        cs = slice(q2 * 1024 + c5 * 512,
                                                   q2 * 1024 + (c5 + 1) * 512)
                                        cl = slice(c5 * 512, (c5 + 1) * 512)
                                        for hi in range(2):
                                            nc.tensor.matmul(
                                                out=sp[hi][:, cl],
                                                lhsT=qh[hi][:, jb * 128:
                                                            (jb + 1) * 128],
                                                rhs=qh[hi][:, cs],
                                                start=True, stop=True)
                                    eb = [ep.tile([128, 1024], F32R, tag="e",
                                                  name=f"eb{_hi}")
                                          for _hi in range(2)]
                                    for hi in range(2):
                                        if jb in VEC_JB:
                                            # Schraudolph exp on DVE: int32
                                            # scratch, then a bit-exact
                                            # gpsimd copy so the e tile has
                                            # an f32r-typed producer (BIR
                                            # verifier requirement for f32r
                                            # matmul inputs)
                                            et = ep.tile(
                                                [128, 1024], I32, tag="et",
                                                name=f"et{hi}")
                                            nc.vector.tensor_scalar(
                                                out=et[:],
                                                in0=sp[hi][:],
                                                scalar1=A_S, scalar2=B_S,
                                                op0=ALU.mult, op1=ALU.add)
                                            nc.gpsimd.tensor_copy(
                                                out=eb[hi][:],
                                                in_=et[:].bitcast(F32R))
                                        else:
                                            nc.scalar.activation(
                                                out=eb[hi][:], in_=sp[hi][:],
                                                func=AF.Exp, scale=SCALE)
                                    for c5 in range(2):
                                        cl = slice(c5 * 512, (c5 + 1) * 512)
                                        for hi in range(2):
                                            h = 2 * pb + hi
                                            nc.tensor.matmul(
                                                out=oT[hi][:, cl],
                                                lhsT=vaug[jb][:, h * VW:
                                                              (h + 1) * VW],
                                                rhs=eb[hi][:, cl],
                                                start=(jb == 0),
                                                stop=(jb == TB - 1))
                                # normalize: ohT = oT[0:64] / Z (Z = row 64);
                                # 1/Z broadcast to 64 partitions via DRAM
                                # bounce (stride-0 reads are DRAM-source only)
                                for hi in range(2):
                                    h = 2 * pb + hi
                                    zr = mp.tile([1, 1024], F32, tag="zr")
                                    nc.vector.reciprocal(
                                        out=zr[:], in_=oT[hi][64:65, :])
                                    nc.sync.dma_start(
                                        out=zscr[h:h + 1, qs], in_=zr[:])
                                    rb = mp.tile([64, 1024], F32, tag="rb")
                                    nc.gpsimd.dma_start(
                                        out=rb[:],
                                        in_=zscr[h:h + 1, qs]
                                        .partition_broadcast(64))
                                    nc.vector.tensor_mul(
                                        out=ohT[pb][hi * 64:hi * 64 + 64, qs],
                                        in0=oT[hi][0:64, :], in1=rb[:])

                    # ------------- Phase C: y_part = ohT^T @ W_out[hg] -----
                    with tc.tile_pool(name="wo", bufs=1) as wop, \
                         tc.tile_pool(name="yp", bufs=3) as yp, \
                         tc.tile_pool(name="psY", bufs=4, space="PSUM") as psY:
                        wot = [wop.tile([128, D], F32R, tag=f"wo{k}",
                                        name=f"wo{k}") for k in range(CB)]
                        for k in range(CB):
                            nc.gpsimd.dma_start(
                                out=wot[k][:],
                                in_=wo_ext[k * 128:(k + 1) * 128, :])
                        for tb in range(TB):
                            for n5 in range(2):
                                ps = psY.tile([128, 512], F32, tag="y")
                                for k in range(CB):
                                    nc.tensor.matmul(
                                        out=ps[:],
                                        lhsT=ohT[k][:, tb * 128:(tb + 1) * 128],
                                        rhs=wot[k][:, n5 * 512:(n5 + 1) * 512],
                                        start=(k == 0), stop=(k == CB - 1))
                                ysb = yp.tile([128, 512], F32, tag="ysb")
                                nc.vector.tensor_add(
                                    out=ysb[:], in0=ps[:],
                                    in1=bias_sb[:, n5 * 512:(n5 + 1) * 512])
                                nc.sync.dma_start(
                                    out=y_ext[tb * 128:(tb + 1) * 128,
                                              n5 * 512:(n5 + 1) * 512],
                                    in_=ysb[:])

    nc.finalize()
    return nc


class _Runner:
    """Caches the finalized Bass program and a jitted shard_map executable so
    repeated kernel() calls skip rebuild/retrace, and so execution can be
    benchmarked with device-resident inputs."""

    def __init__(self, reps=1):
        import jax
        from jax.sharding import Mesh, PartitionSpec
        from jax.experimental.shard_map import shard_map
        from concourse import mybir as _mybir
        from concourse.bass2jax import (
            _bass_exec_p, install_neuronx_cc_hook, partition_id_tensor)

        install_neuronx_cc_hook()
        nc = _build_program(reps=reps)
        self.nc = nc

        in_names, out_names, out_avals = [], [], []
        partition_name = (nc.partition_id_tensor.name
                          if nc.partition_id_tensor else None)
        for alloc in nc.m.functions[0].allocations:
            if not isinstance(alloc, _mybir.MemoryLocationSet):
                continue
            name = alloc.memorylocations[0].name
            if alloc.kind == "ExternalInput":
                if name != partition_name:
                    in_names.append(name)
            elif alloc.kind == "ExternalOutput":
                out_names.append(name)
                out_avals.append(jax.core.ShapedArray(
                    tuple(alloc.tensor_shape), _mybir.dt.np(alloc.dtype)))
        self.in_names = list(in_names)
        self.out_names = out_names
        self.out_avals = out_avals
        all_in_names = in_names + out_names
        if partition_name is not None:
            all_in_names = all_in_names + [partition_name]

        def _body(*args):
            operands = list(args)
            if partition_name is not None:
                operands.append(partition_id_tensor())
            outs = _bass_exec_p.bind(
                *operands,
                out_avals=tuple(out_avals),
                in_names=tuple(all_in_names),
                out_names=tuple(out_names),
                lowering_input_output_aliases=(),
                sim_require_finite=False,
                sim_require_nnan=False,
                nc=nc,
            )
            return tuple(outs)

        self._body = _body

        devices = jax.devices()[:N_CORES]
        mesh = Mesh(np.asarray(devices), ("core",))
        self.mesh = mesh
        n_in = len(in_names) + len(out_names)
        self.sharded = jax.jit(shard_map(
            _body, mesh=mesh,
            in_specs=(PartitionSpec("core"),) * n_in,
            out_specs=(PartitionSpec("core"),) * len(out_names),
            check_rep=False))
        self.zero_outs = [
            np.zeros((N_CORES * a.shape[0], *a.shape[1:]), a.dtype)
            for a in out_avals]

    def run_concat(self, concat_inputs):
        """concat_inputs: list matching in_names, each [8*dim0, ...]."""
        return self.sharded(*concat_inputs, *self.zero_outs)


def _get_runner():
    if "runner" not in _CACHE:
        _CACHE["runner"] = _Runner()
    return _CACHE["runner"]


def _shard_inputs(x, W_qkv, W_out, b_out):
    """Build the concatenated per-core input arrays (order = in_names).
    core c: batch c//2, head-group hg = c%2 (channels hg*512:(hg+1)*512)."""
    xts, wqs, wos, bos = [], [], [], []
    zeros_b = np.zeros_like(b_out)
    for c in range(N_CORES):
        b, hg = c // 2, c % 2
        xts.append(np.ascontiguousarray(x[b].T))            # [D, N]
        wqs.append(W_qkv[:, hg * DG:(hg + 1) * DG])          # [D, 512]
        wos.append(W_out[hg * DG:(hg + 1) * DG, :])          # [512, D]
        bos.append(b_out if hg == 0 else zeros_b)
    by_name = {
        "xT": np.concatenate(xts, axis=0),
        "w_qkv": np.concatenate(wqs, axis=0),
        "w_out": np.concatenate(wos, axis=0),
        "b_out": np.concatenate(bos, axis=0),
    }
    return by_name


def kernel(x, W_qkv, W_out, b_out):
    x = np.asarray(x, dtype=np.float32)
    W_qkv = np.asarray(W_qkv, dtype=np.float32)
    W_out = np.asarray(W_out, dtype=np.float32)
    b_out = np.asarray(b_out, dtype=np.float32).reshape(1, D)

    runner = _get_runner()
    by_name = _shard_inputs(x, W_qkv, W_out, b_out)
    concat_in = [by_name[n] for n in runner.in_names]
    outs = runner.run_concat(concat_in)

    y_all = np.asarray(outs[runner.out_names.index("y")])
    y_all = y_all.reshape(N_CORES, N, D)
    # head-group partials: out[b] = y[2b] + y[2b+1] (bias rides on hg=0)
    out = np.empty((B, N, D), np.float32)
    for b in range(B):
        out[b] = y_all[2 * b] + y_all[2 * b + 1]
    return out
